# revision 6
# baseline (speedup 1.0000x reference)
"""Deformable-ROI bilinear feature gather (KeypPointBboxNet) on 8 TRN2 cores.

Strategy: feat_map sharded on batch (one image per core, HWC fp16 so a
pixel's 256 channels are 512 contiguous bytes); rois/offsets routed
host-side to the core holding their image. Per core:
  - coordinate + weight pipeline in [128, S] layout on DVE (f32),
  - indices shuffled into the 16-partition-wrapped gather layout with
    top/bottom rows interleaved per slot, so ONE dma_gather per chunk
    fetches all 4 bilinear taps of 128x CH points (fp16: 1KB/descriptor),
  - combine alternates two paths per slot to spread load:
      PE path:  4 accumulating diag(w_k) @ v_k matmuls into PSUM
                (diag matrices are loop-invariant, built once on DVE),
                ACT evacuates PSUM -> fp16 out slot;
      DVE path: factored lerp via ACT scale + DVE ts/tt fp16 fast modes,
  - per-chunk fp16 stores overlap the next chunk's gather/compute.
fp16 features give ~1e-3 rel err vs the 2e-2 gate and halve both DMA
bytes and DVE elementwise cost vs f32.
"""

import math

import numpy as np

B, C, H, W = 8, 256, 128, 128
N_ROIS, NUM_POINT, STRIDE = 2048, 9, 8
NCORES = 8
CH = 2  # slots (of 128 points) per chunk; 256*CH gather descriptors per
        # call must stay <= 1024 (SWDGE descriptor-ring carveout) or the
        # call dies on HW with an opaque INTERNAL error. Keep <= 512 so two
        # calls fit in the ring and desc-gen overlaps the previous transfer.
# fm rows addressable by gathers: idx_bot can reach H*W + W - 1 = 16511 and
# each gather reads 2 pixels -> pad the image to 16640 rows of zeros.
FM_ROWS = H * W + 2 * W
FM_VIEW_ROWS = FM_ROWS - 1

_PROGRAM_CACHE: dict[int, object] = {}
_PE_PATTERN = lambda s: s % 2 == 0  # which slots take the PE combine path


def _build_program(S: int, iters: int = 1):
    import concourse.bacc as bacc
    import concourse.mybir as mybir
    import concourse.tile as tile
    from concourse.bass_types import AP

    f16 = mybir.dt.float16
    f32 = mybir.dt.float32
    i16 = mybir.dt.int16
    op = mybir.AluOpType
    ACT_COPY = mybir.ActivationFunctionType.Copy

    chunks = []
    a = 0
    while a < S:
        b = min(a + CH, S)
        chunks.append((a, b))
        a = b

    nc = bacc.Bacc("TRN2", target_bir_lowering=False, debug=False, num_devices=NCORES)
    fm_t = nc.dram_tensor("fm", [FM_ROWS, C], f16, kind="ExternalInput")
    pt_t = nc.dram_tensor("pt", [128, S * 6], f32, kind="ExternalInput")
    id_t = nc.dram_tensor("ident", [128, 128], f16, kind="ExternalInput")
    out_t = nc.dram_tensor("out", [128, S * C], f16, kind="ExternalOutput")

    # fm viewed as overlapping [row, 2*C] rows with stride C (one gathered
    # element = pixels (h,w),(h,w+1) = 1KB fp16).
    fm_gather_ap = AP(fm_t, 0, [[C, FM_VIEW_ROWS], [1, 2 * C]])

    with tile.TileContext(nc) as tc:
        with (
            tc.tile_pool(name="const", bufs=1) as cpool,
            tc.tile_pool(name="gath", bufs=2) as gpool,
            tc.tile_pool(name="work", bufs=3) as wpool,
            tc.tile_pool(name="outp", bufs=2) as opool,
            tc.tile_pool(name="psum", bufs=4, space="PSUM") as ppool,
        ):
            p128 = cpool.tile([128, S * 6], f32)
            nc.sync.dma_start(p128[:], pt_t[:])
            ident = cpool.tile([128, 128], f16)
            nc.sync.dma_start(ident[:], id_t[:])

            v = p128[:].rearrange("p (q f) -> p q f", f=6)

            def coord_chain(axis):
                """cif = clip(floor(coord),0,127)+16 (f32), d = edge-masked frac."""
                lo = v[:, :, 0 + axis]
                hi = v[:, :, 2 + axis]
                off = v[:, :, 4 + axis]
                w0 = wpool.tile([128, S], f32, tag=f"w0{axis}")
                nc.vector.tensor_tensor(w0[:], hi, lo, op.subtract)
                sx = wpool.tile([128, S], f32, tag=f"sx{axis}")
                nc.vector.tensor_scalar(sx[:], w0[:], 1.0, 0.1 / STRIDE, op.add, op.mult)
                asum = wpool.tile([128, S], f32, tag=f"as{axis}")
                nc.vector.tensor_tensor(asum[:], lo, hi, op.add)
                ax = wpool.tile([128, S], f32, tag=f"ax{axis}")
                nc.vector.tensor_scalar(ax[:], asum[:], 0.5 / STRIDE, 16.0, op.mult, op.add)
                ixs = wpool.tile([128, S], f32, tag=f"ix{axis}")
                nc.vector.tensor_tensor(ixs[:], off, sx[:], op.mult)
                nc.vector.tensor_tensor(ixs[:], ixs[:], ax[:], op.add)  # coord+16
                ci = wpool.tile([128, S], mybir.dt.int32, tag=f"ci{axis}")
                nc.vector.tensor_copy(ci[:], ixs[:])
                cif = wpool.tile([128, S], f32, tag=f"cf{axis}")
                nc.vector.tensor_copy(cif[:], ci[:])
                gt = wpool.tile([128, S], f32, tag=f"gt{axis}")
                nc.vector.tensor_tensor(gt[:], cif[:], ixs[:], op.is_gt)
                nc.vector.tensor_tensor(cif[:], cif[:], gt[:], op.subtract)  # floor+16
                nc.vector.tensor_scalar(cif[:], cif[:], 143.0, 16.0, op.min, op.max)
                d = cpool.tile([128, S], f32, tag=f"d{axis}")
                nc.vector.tensor_tensor(d[:], ixs[:], cif[:], op.subtract)
                m = wpool.tile([128, S], f32, tag=f"m{axis}")
                nc.vector.tensor_scalar(m[:], cif[:], 143.0, None, op.is_lt)
                nc.vector.tensor_tensor(d[:], d[:], m[:], op.mult)
                return cif, d

            ccx, lw = coord_chain(0)
            ccy, lh = coord_chain(1)

            # --- flat row index, top and bottom, in [128, S] then shuffled to
            # the [16, 16S] wrapped+interleaved gather layout.
            idxf = wpool.tile([128, S], f32, tag="idxf")
            nc.vector.scalar_tensor_tensor(idxf[:], ccy[:], float(W), ccx[:], op.mult, op.add)
            idxt_f = wpool.tile([128, S], f32, tag="idxtf")
            nc.vector.tensor_scalar(idxt_f[:], idxf[:], -(16.0 * W + 16.0), None, op.add)
            idxb_f = wpool.tile([128, S], f32, tag="idxbf")
            nc.vector.tensor_scalar(idxb_f[:], idxf[:], -(16.0 * W + 16.0) + W, None, op.add)
            idxt = wpool.tile([128, S], i16, tag="idxt16")
            nc.vector.tensor_copy(idxt[:], idxt_f[:])
            idxb = wpool.tile([128, S], i16, tag="idxb16")
            nc.vector.tensor_copy(idxb[:], idxb_f[:])

            # idx16 col layout: 16*s + 8*t + g  (t=0 top / 1 bottom, g=p//16)
            idx16 = cpool.tile([128, 16 * S], i16)
            idx16v = idx16[:].rearrange("p (s t g) -> p s t g", t=2, g=8)
            for g in range(8):
                nc.sync.dma_start(idx16v[0:16, :, 0, g], idxt[g * 16 : (g + 1) * 16, :])
                nc.sync.dma_start(idx16v[0:16, :, 1, g], idxb[g * 16 : (g + 1) * 16, :])
            nc.sync.dma_start(idx16[16:32], idx16[0:16])
            nc.sync.dma_start(idx16[32:64], idx16[0:32])
            nc.sync.dma_start(idx16[64:128], idx16[0:64])

            # --- bilinear weights; ch for the DVE path, 4 products for PE.
            ch = cpool.tile([128, S], f32)
            nc.vector.tensor_scalar(ch[:], lh[:], -1.0, 1.0, op.mult, op.add)
            w22 = cpool.tile([128, S], f32)
            nc.vector.tensor_tensor(w22[:], lh[:], lw[:], op.mult)
            s1 = wpool.tile([128, S], f32, tag="s1")
            nc.vector.tensor_tensor(s1[:], lh[:], lw[:], op.add)
            w12 = cpool.tile([128, S], f32)
            nc.vector.tensor_tensor(w12[:], lw[:], w22[:], op.subtract)
            w21 = cpool.tile([128, S], f32)
            nc.vector.tensor_tensor(w21[:], lh[:], w22[:], op.subtract)
            w11 = cpool.tile([128, S], f32)
            nc.vector.tensor_tensor(w11[:], w22[:], s1[:], op.subtract)
            nc.vector.tensor_scalar(w11[:], w11[:], 1.0, None, op.add)

            # --- loop-invariant diag(w_k) stacks for the PE-path slots.
            pe_slot = [_PE_PATTERN(s) for s in range(S)]
            wmats = [w11, w12, w21, w22]
            diags = cpool.tile([128, 4 * S, 128], f16)
            for s in range(S):
                if not pe_slot[s]:
                    continue
                for k in range(4):
                    nc.vector.tensor_scalar(
                        diags[:, 4 * s + k, :], ident[:], wmats[k][:, s : s + 1],
                        None, op.mult,
                    )

            out_v = out_t[:].rearrange("p (s c) -> p s c", c=C)

            for _it in range(iters):
                for (a, b) in chunks:
                    bsz = b - a
                    gt = gpool.tile([128, 2 * bsz, 2 * C], f16, tag=f"gt{bsz}")
                    nc.gpsimd.dma_gather(
                        gt[:], fm_gather_ap, idx16[:, 16 * a : 16 * b],
                        256 * bsz, 256 * bsz, 2 * C, elem_step=C,
                    )
                    outc = opool.tile([128, bsz, C], f16, tag=f"oc{bsz}")
                    for s in range(a, b):
                        ds = s - a
                        top = gt[:, 2 * ds, :]
                        bot = gt[:, 2 * ds + 1, :]
                        if pe_slot[s]:
                            ps = ppool.tile([128, C], f32, tag="ps")
                            nc.tensor.matmul(
                                ps[:], diags[:, 4 * s + 0, :], top[:, 0:C],
                                start=True, stop=False)
                            nc.tensor.matmul(
                                ps[:], diags[:, 4 * s + 1, :], top[:, C : 2 * C],
                                start=False, stop=False)
                            nc.tensor.matmul(
                                ps[:], diags[:, 4 * s + 2, :], bot[:, 0:C],
                                start=False, stop=False)
                            nc.tensor.matmul(
                                ps[:], diags[:, 4 * s + 3, :], bot[:, C : 2 * C],
                                start=False, stop=True)
                            nc.scalar.activation(outc[:, ds, :], ps[:], ACT_COPY)
                        else:
                            t1 = wpool.tile([128, 2 * C], f16, tag="t1")
                            nc.scalar.activation(
                                t1[:], top, ACT_COPY, bias=0.0,
                                scale=ch[:, s : s + 1],
                            )
                            m2 = wpool.tile([128, 2 * C], f16, tag="m2")
                            nc.vector.tensor_scalar(
                                m2[:], bot, lh[:, s : s + 1], None, op.mult)
                            st = wpool.tile([128, 2 * C], f16, tag="st")
                            nc.vector.tensor_tensor(st[:], t1[:], m2[:], op.add)
                            d = wpool.tile([128, C], f16, tag="dd")
                            nc.vector.tensor_tensor(
                                d[:], st[:, C : 2 * C], st[:, 0:C], op.subtract)
                            e = wpool.tile([128, C], f16, tag="ee")
                            nc.vector.tensor_scalar(
                                e[:], d[:], lw[:, s : s + 1], None, op.mult)
                            nc.vector.tensor_tensor(
                                outc[:, ds, :], e[:], st[:, 0:C], op.add)
                    nc.sync.dma_start(out_v[:, a:b, :], outc[:])

    nc.compile()
    return nc


def _get_program(S: int):
    if S not in _PROGRAM_CACHE:
        _PROGRAM_CACHE[S] = _build_program(S)
    return _PROGRAM_CACHE[S]


_IDENT = None


def _host_prep(feat_map, rois, offset, num_point):
    """Route rois by batch index; build per-core inputs."""
    global _IDENT
    if _IDENT is None:
        _IDENT = np.eye(128, dtype=np.float16)
    bidx = rois[:, 0].astype(np.int32)
    ids = [np.nonzero(bidx == b)[0] for b in range(B)]
    cap = max(len(i) for i in ids)
    S = math.ceil(max(cap * num_point, 1) / 128)

    NP = S * 128
    in_maps = []
    for b in range(B):
        fmb = feat_map[b].transpose(1, 2, 0).reshape(H * W, C)
        fm_full = np.zeros((FM_ROWS, C), np.float16)
        fm_full[: H * W] = fmb
        ptdata = np.zeros((NP, 6), np.float32)
        idl = ids[b]
        nb = len(idl)
        if nb:
            r = rois[idl]
            off = offset[idl].reshape(nb, num_point, 2)
            npts = nb * num_point
            ptdata[:npts, 0] = np.repeat(r[:, 1], num_point)
            ptdata[:npts, 1] = np.repeat(r[:, 2], num_point)
            ptdata[:npts, 2] = np.repeat(r[:, 3], num_point)
            ptdata[:npts, 3] = np.repeat(r[:, 4], num_point)
            ptdata[:npts, 4] = off[:, :, 0].reshape(-1)
            ptdata[:npts, 5] = off[:, :, 1].reshape(-1)
        pt128 = np.ascontiguousarray(
            ptdata.reshape(S, 128, 6).transpose(1, 0, 2)
        ).reshape(128, S * 6)
        in_maps.append({"fm": fm_full, "pt": pt128, "ident": _IDENT})
    return ids, S, in_maps


def _host_unshard(results, ids, S, num_point, n):
    out_full = np.zeros((n, num_point, C), np.float32)
    for b in range(B):
        nb = len(ids[b])
        if not nb:
            continue
        o = results[b]["out"].reshape(128, S, C).transpose(1, 0, 2).reshape(S * 128, C)
        out_full[ids[b]] = o[: nb * num_point].astype(np.float32).reshape(nb, num_point, C)
    return out_full


def kernel(feat_map, rois, offset, stride, num_point, _collect=None):
    from concourse.bass_utils import run_bass_kernel_spmd

    feat_map = np.asarray(feat_map, np.float32)
    rois = np.asarray(rois, np.float32)
    offset = np.asarray(offset, np.float32)
    stride = int(stride)
    num_point = int(num_point)
    assert feat_map.shape == (B, C, H, W), feat_map.shape
    assert stride == STRIDE and num_point == NUM_POINT

    ids, S, in_maps = _host_prep(feat_map, rois, offset, num_point)
    nc = _get_program(S)
    res = run_bass_kernel_spmd(nc, in_maps, core_ids=list(range(NCORES)),
                               **(_collect.pop("spmd_kwargs", {}) if _collect else {}))
    if _collect is not None:
        _collect["res"] = res
    return _host_unshard(res.results, ids, S, num_point, rois.shape[0])
